# revision 31
# baseline (speedup 1.0000x reference)
"""LIF spike scan kernel for Trainium2 (8 NeuronCores, data-parallel).

Reference computation (per element, scanned over t):
    mem = mem * 0.2 * (1 - spk) + x[t]
    spk = (mem > 0.5)

Carry formulation used here (v = mem * (mem <= 0.5), the post-reset membrane):
    m   = (v * 0.2) + x[t]        -> one DVE scalar_tensor_tensor
    spk = relu(sign(m - 0.5))     -> two ACT ops (exact 0/1 in fp32)
    v   = (m <= 0.5) * m          -> one DVE scalar_tensor_tensor

All arithmetic is fp32 and bit-identical to the jax reference: multiplying by
the exact constants {0.0, 1.0, 0.2} commutes with the reference's rounding.

Sharding: x is [T=16, B=64, C=128, H=32, W=32]; the scan is elementwise over
the 8M spatial elements, so each core takes a contiguous 1/8 slice of the
flattened B*C*H*W axis (8 batches per core) viewed as [T, 128, 8192].
"""

import numpy as np

T = 16
SPATIAL = 64 * 128 * 32 * 32  # 8388608
N_CORES = 8
NPC = SPATIAL // N_CORES      # 1048576 elements per core per timestep
P = 128                       # SBUF partitions
Q = NPC // P                  # 8192 free-dim columns per core
F = 2048                      # free-dim tile size
DECAY = 0.2
THRESH = 0.5

_cache = {}

# Set by test harness to request an NTFF trace / HW timing.
TRACE = False


def _build():
    from contextlib import ExitStack

    import concourse.bacc as bacc
    import concourse.tile as tile
    from concourse import mybir

    f32 = mybir.dt.float32
    u8 = mybir.dt.uint8
    Alu = mybir.AluOpType
    Act = mybir.ActivationFunctionType

    nc = bacc.Bacc("TRN2", target_bir_lowering=False, debug=False)
    x_d = nc.dram_tensor("x", [T, P, Q], f32, kind="ExternalInput").ap()
    # Spikes are exactly 0/1, so ship them as uint8 (4x less store traffic)
    # and widen to fp32 on the host.
    o_d = nc.dram_tensor("spk", [T, P, Q], u8, kind="ExternalOutput").ap()

    # Register -THRESH as a const AP (like Bass.__init__ does for 0.0/1.0):
    # written once before the Tile region + barrier, so activation bias
    # reads are untracked and add no per-instruction semaphore wait (the
    # Activation ISA slot only fits one wait).
    _bias = nc.alloc_sbuf_tensor("const-f32-negthresh", [128, 1], f32)
    nc.gpsimd.memset(_bias.ap(), -THRESH)
    nc.const_aps.aps[(f32, -THRESH)] = _bias.ap()
    nc.all_engine_barrier()

    with tile.TileContext(nc) as tc, ExitStack() as ctx:
        xpool = ctx.enter_context(tc.tile_pool(name="xin", bufs=8))
        vpool = ctx.enter_context(tc.tile_pool(name="vre", bufs=3))
        spool = ctx.enter_context(tc.tile_pool(name="sgn", bufs=3))
        opool = ctx.enter_context(tc.tile_pool(name="out", bufs=4))

        for q0 in range(0, Q, F):
            v = None
            for t in range(T):
                xt = xpool.tile([P, F], f32)
                nc.sync.dma_start(xt[:], x_d[t, :, q0 : q0 + F])
                # mem update in place on the freshly-loaded x tile:
                # m = (v * DECAY) + x[t]; at t=0, m = x[0] exactly.
                m = xt
                if v is not None:
                    nc.vector.scalar_tensor_tensor(
                        m[:], v[:], DECAY, xt[:], op0=Alu.mult, op1=Alu.add
                    )
                s = spool.tile([P, F], f32)
                nc.scalar.activation(s[:], m[:], Act.Sign, bias=-THRESH)
                o = opool.tile([P, F], u8)
                nc.scalar.activation(o[:], s[:], Act.Relu)
                nc.sync.dma_start(o_d[t, :, q0 : q0 + F], o[:])
                if t < T - 1:
                    vn = vpool.tile([P, F], f32)
                    nc.vector.scalar_tensor_tensor(
                        vn[:], m[:], THRESH, m[:], op0=Alu.is_le, op1=Alu.mult
                    )
                    v = vn
    nc.compile()
    return nc


def kernel(x: np.ndarray) -> np.ndarray:
    from concourse.bass_utils import run_bass_kernel_spmd

    if "nc" not in _cache:
        _cache["nc"] = _build()
    nc = _cache["nc"]

    x = np.ascontiguousarray(x, dtype=np.float32).reshape(T, N_CORES, NPC)
    in_maps = [
        {"x": np.ascontiguousarray(x[:, i]).reshape(T, P, Q)} for i in range(N_CORES)
    ]
    res = run_bass_kernel_spmd(
        nc, in_maps, core_ids=list(range(N_CORES)), trace=TRACE
    )
    _cache["last_results"] = res
    out = np.stack(
        [np.asarray(r["spk"]).astype(np.float32).reshape(T, NPC) for r in res.results],
        axis=1,
    )
    return out.reshape(T, 64, 128, 32, 32)



# revision 33
# speedup vs baseline: 1.0438x; 1.0438x over previous
"""LIF spike scan kernel for Trainium2 (8 NeuronCores, data-parallel).

Reference computation (per element, scanned over t):
    mem = mem * 0.2 * (1 - spk) + x[t]
    spk = (mem > 0.5)

Carry formulation used here (v = mem * (mem <= 0.5), the post-reset membrane):
    m   = (v * 0.2) + x[t]        -> one DVE scalar_tensor_tensor
    spk = relu(sign(m - 0.5))     -> two ACT ops (exact 0/1 in fp32)
    v   = (m <= 0.5) * m          -> one DVE scalar_tensor_tensor

All arithmetic is fp32 and bit-identical to the jax reference: multiplying by
the exact constants {0.0, 1.0, 0.2} commutes with the reference's rounding.

Sharding: x is [T=16, B=64, C=128, H=32, W=32]; the scan is elementwise over
the 8M spatial elements, so each core takes a contiguous 1/8 slice of the
flattened B*C*H*W axis (8 batches per core) viewed as [T, 128, 8192].
"""

import numpy as np

T = 16
SPATIAL = 64 * 128 * 32 * 32  # 8388608
N_CORES = 8
NPC = SPATIAL // N_CORES      # 1048576 elements per core per timestep
P = 128                       # SBUF partitions
Q = NPC // P                  # 8192 free-dim columns per core
F = 2048                      # free-dim tile size
DECAY = 0.2
THRESH = 0.5

_cache = {}

# Set by test harness to request an NTFF trace / HW timing.
TRACE = False


def _build():
    from contextlib import ExitStack

    import concourse.bacc as bacc
    import concourse.tile as tile
    from concourse import mybir

    f32 = mybir.dt.float32
    u8 = mybir.dt.uint8
    Alu = mybir.AluOpType
    Act = mybir.ActivationFunctionType

    nc = bacc.Bacc("TRN2", target_bir_lowering=False, debug=False)
    x_d = nc.dram_tensor("x", [T, P, Q], f32, kind="ExternalInput").ap()
    # Spikes are exactly 0/1, so ship them as uint8 (4x less store traffic)
    # and widen to fp32 on the host.
    o_d = nc.dram_tensor("spk", [T, P, Q], u8, kind="ExternalOutput").ap()

    # Register -THRESH as a const AP (like Bass.__init__ does for 0.0/1.0):
    # written once before the Tile region + barrier, so activation bias
    # reads are untracked and add no per-instruction semaphore wait (the
    # Activation ISA slot only fits one wait).
    _bias = nc.alloc_sbuf_tensor("const-f32-negthresh", [128, 1], f32)
    nc.gpsimd.memset(_bias.ap(), -THRESH)
    nc.const_aps.aps[(f32, -THRESH)] = _bias.ap()
    nc.all_engine_barrier()

    with tile.TileContext(nc) as tc, ExitStack() as ctx:
        xpool = ctx.enter_context(tc.tile_pool(name="xin", bufs=10))
        vpool = ctx.enter_context(tc.tile_pool(name="vre", bufs=4))
        spool = ctx.enter_context(tc.tile_pool(name="sgn", bufs=4))
        opool = ctx.enter_context(tc.tile_pool(name="out", bufs=6))

        # Two q-tile chains interleaved per pass: while one chain's t-step
        # waits on anything (input DMA, tile WAR), DVE and ACT stay busy on
        # the other chain. Single-chain processing left ~30us of chain
        # restart / dependency gaps on the DVE critical path.
        for pair in range(Q // (2 * F)):
            qs = [(2 * pair + i) * F for i in range(2)]
            v = {q0: None for q0 in qs}
            for t in range(T):
                for q0 in qs:
                    xt = xpool.tile([P, F], f32, name="xt")
                    nc.sync.dma_start(xt[:], x_d[t, :, q0 : q0 + F])
                    # mem update in place on the freshly-loaded x tile:
                    # m = (v * DECAY) + x[t]; at t=0, m = x[0] exactly.
                    m = xt
                    if v[q0] is not None:
                        nc.vector.scalar_tensor_tensor(
                            m[:], v[q0][:], DECAY, xt[:],
                            op0=Alu.mult, op1=Alu.add,
                        )
                    s = spool.tile([P, F], f32, name="s")
                    nc.scalar.activation(s[:], m[:], Act.Sign, bias=-THRESH)
                    o = opool.tile([P, F], u8, name="o")
                    nc.scalar.activation(o[:], s[:], Act.Relu)
                    nc.sync.dma_start(o_d[t, :, q0 : q0 + F], o[:])
                    if t < T - 1:
                        vn = vpool.tile([P, F], f32, name="vn")
                        nc.vector.scalar_tensor_tensor(
                            vn[:], m[:], THRESH, m[:],
                            op0=Alu.is_le, op1=Alu.mult,
                        )
                        v[q0] = vn
    nc.compile()
    return nc


def kernel(x: np.ndarray) -> np.ndarray:
    from concourse.bass_utils import run_bass_kernel_spmd

    if "nc" not in _cache:
        _cache["nc"] = _build()
    nc = _cache["nc"]

    x = np.ascontiguousarray(x, dtype=np.float32).reshape(T, N_CORES, NPC)
    in_maps = [
        {"x": np.ascontiguousarray(x[:, i]).reshape(T, P, Q)} for i in range(N_CORES)
    ]
    res = run_bass_kernel_spmd(
        nc, in_maps, core_ids=list(range(N_CORES)), trace=TRACE
    )
    _cache["last_results"] = res
    out = np.stack(
        [np.asarray(r["spk"]).astype(np.float32).reshape(T, NPC) for r in res.results],
        axis=1,
    )
    return out.reshape(T, 64, 128, 32, 32)



# revision 34
# speedup vs baseline: 1.2009x; 1.1505x over previous
"""LIF spike scan kernel for Trainium2 (8 NeuronCores, data-parallel).

Reference computation (per element, scanned over t):
    mem = mem * 0.2 * (1 - spk) + x[t]
    spk = (mem > 0.5)

Carry formulation (v = mem * (mem <= 0.5), the post-reset membrane):
    m   = (v * 0.2) + x[t]            -> DVE scalar_tensor_tensor (split in
                                         two column ranges, one per v half)
    spk = (m > 0.5)                   -> ONE ACT op: u8 = sign(m - 0.5);
                                         the fp32->u8 write saturates, so
                                         -1 clamps to 0 and spikes land as
                                         exact {0,1} (hardware-verified)
    v   = (m <= 0.5) * m, split:
          cols [0,FD)   : DVE scalar_tensor_tensor
          cols [FD,F)   : b = (m <= 0.5) on DVE tensor_scalar (2x mode),
                          v = b * m on Pool tensor_tensor (Multiply is the
                          one elementwise op neuronxcc accepts on Pool)

All membrane arithmetic is fp32 and bit-identical to the jax reference.

Per-iteration engine costs (TimelineSim model, F=2048, FD=384):
    DVE : m-stt 2253 + v-stt(384) 460 + is_le(1664) 925  = ~3640ns
    Pool: v-mult(1664) at 0.42 Q7 efficiency             = ~3400ns
    ACT : sign->u8                                       = ~1892ns
    DMA : x in 2913 + u8 spikes out 728                  = ~3641ns <- bound
All engines sit at or under the DMA floor (~233us); the previous
checkpoint (280us) was DVE-bound with both stt ops plus nothing
offloadable. The one-op u8 spike is what frees ACT to absorb nothing
and DVE to give the large v half to Pool.

Two q-tile chains are interleaved so the cross-engine hop in the Pool
half's recurrence (m -> is_le -> Pool mult -> next m) is latency-hidden:
chain-step latency ~3.2us < 2x the 3.64us iteration budget.

Sharding: x is [T=16, B=64, C=128, H=32, W=32]; the scan is elementwise
over the 8M spatial elements, so each core takes a contiguous 1/8 slice
of the flattened B*C*H*W axis viewed as [T, 128, 8192]. Spikes ship as
u8 (4x less store traffic) and widen to fp32 on the host.
"""

import numpy as np

T = 16
SPATIAL = 64 * 128 * 32 * 32  # 8388608
N_CORES = 8
NPC = SPATIAL // N_CORES      # 1048576 elements per core per timestep
P = 128                       # SBUF partitions
Q = NPC // P                  # 8192 free-dim columns per core
F = 2048                      # free-dim tile size
FD = 384                      # v columns computed on DVE; rest go to Pool
DECAY = 0.2
THRESH = 0.5

_cache = {}

# Set by test harness to request an NTFF trace / HW timing.
TRACE = False


def _build():
    from contextlib import ExitStack

    import concourse.bacc as bacc
    import concourse.tile as tile
    from concourse import mybir

    f32 = mybir.dt.float32
    u8 = mybir.dt.uint8
    Alu = mybir.AluOpType
    Act = mybir.ActivationFunctionType

    nc = bacc.Bacc("TRN2", target_bir_lowering=False, debug=False)
    x_d = nc.dram_tensor("x", [T, P, Q], f32, kind="ExternalInput").ap()
    o_d = nc.dram_tensor("spk", [T, P, Q], u8, kind="ExternalOutput").ap()

    # Register -THRESH as a const AP (like Bass.__init__ does for 0.0/1.0):
    # written once before the Tile region + barrier, so activation bias
    # reads are untracked and add no per-instruction semaphore wait (the
    # Activation ISA slot only fits one wait).
    _bias = nc.alloc_sbuf_tensor("const-f32-negthresh", [128, 1], f32)
    nc.gpsimd.memset(_bias.ap(), -THRESH)
    nc.const_aps.aps[(f32, -THRESH)] = _bias.ap()
    nc.all_engine_barrier()

    with tile.TileContext(nc) as tc, ExitStack() as ctx:
        xpool = ctx.enter_context(tc.tile_pool(name="xin", bufs=9))
        dpool = ctx.enter_context(tc.tile_pool(name="vd", bufs=5))
        ppool = ctx.enter_context(tc.tile_pool(name="vp", bufs=5))
        bpool = ctx.enter_context(tc.tile_pool(name="ble", bufs=4))
        opool = ctx.enter_context(tc.tile_pool(name="out", bufs=6))

        # Two q-tile chains interleaved per pass: while one chain's Pool
        # multiply is in flight, DVE runs the other chain.
        for pair in range(Q // (2 * F)):
            qs = [(2 * pair + i) * F for i in range(2)]
            v = {q0: (None, None) for q0 in qs}  # (v_dve, v_pool) tiles
            for t in range(T):
                for q0 in qs:
                    xt = xpool.tile([P, F], f32, name="xt")
                    nc.sync.dma_start(xt[:], x_d[t, :, q0 : q0 + F])
                    # mem update in place on the freshly-loaded x tile, one
                    # stt per carry half; at t=0, m = x[0] exactly.
                    m = xt
                    vd, vp = v[q0]
                    if vd is not None:
                        nc.vector.scalar_tensor_tensor(
                            m[:, FD:F], vp[:], DECAY, xt[:, FD:F],
                            op0=Alu.mult, op1=Alu.add,
                        )
                        nc.vector.scalar_tensor_tensor(
                            m[:, 0:FD], vd[:], DECAY, xt[:, 0:FD],
                            op0=Alu.mult, op1=Alu.add,
                        )
                    # spike in one ACT op: u8 = sign(m - 0.5), saturating
                    # write clamps -1 to 0.
                    o = opool.tile([P, F], u8, name="o")
                    nc.scalar.activation(o[:], m[:], Act.Sign, bias=-THRESH)
                    nc.sync.dma_start(o_d[t, :, q0 : q0 + F], o[:])
                    if t < T - 1:
                        b = bpool.tile([P, F - FD], f32, name="b")
                        nc.vector.tensor_scalar(
                            b[:], m[:, FD:F], THRESH, None, op0=Alu.is_le
                        )
                        vdn = dpool.tile([P, FD], f32, name="vdn")
                        nc.vector.scalar_tensor_tensor(
                            vdn[:], m[:, 0:FD], THRESH, m[:, 0:FD],
                            op0=Alu.is_le, op1=Alu.mult,
                        )
                        vpn = ppool.tile([P, F - FD], f32, name="vpn")
                        nc.gpsimd.tensor_tensor(
                            vpn[:], b[:], m[:, FD:F], op=Alu.mult
                        )
                        v[q0] = (vdn, vpn)
    nc.compile()
    return nc


def kernel(x: np.ndarray) -> np.ndarray:
    from concourse.bass_utils import run_bass_kernel_spmd

    if "nc" not in _cache:
        _cache["nc"] = _build()
    nc = _cache["nc"]

    x = np.ascontiguousarray(x, dtype=np.float32).reshape(T, N_CORES, NPC)
    in_maps = [
        {"x": np.ascontiguousarray(x[:, i]).reshape(T, P, Q)} for i in range(N_CORES)
    ]
    res = run_bass_kernel_spmd(
        nc, in_maps, core_ids=list(range(N_CORES)), trace=TRACE
    )
    _cache["last_results"] = res
    out = np.stack(
        [np.asarray(r["spk"]).astype(np.float32).reshape(T, NPC) for r in res.results],
        axis=1,
    )
    return out.reshape(T, 64, 128, 32, 32)


# revision 36
# speedup vs baseline: 1.2097x; 1.0074x over previous
"""LIF spike scan kernel for Trainium2 (8 NeuronCores, data-parallel).

Reference computation (per element, scanned over t):
    mem = mem * 0.2 * (1 - spk) + x[t]
    spk = (mem > 0.5)

Carry formulation (v = mem * (mem <= 0.5), the post-reset membrane):
    m   = (v * 0.2) + x[t]            -> DVE scalar_tensor_tensor (split in
                                         two column ranges, one per v half)
    spk = (m > 0.5)                   -> ONE ACT op: u8 = sign(m - 0.5);
                                         the fp32->u8 write saturates, so
                                         -1 clamps to 0 and spikes land as
                                         exact {0,1} (hardware-verified)
    v   = (m <= 0.5) * m, split:
          cols [0,FD)   : DVE scalar_tensor_tensor
          cols [FD,F)   : b = (m <= 0.5) on DVE tensor_scalar (2x mode),
                          v = b * m on Pool tensor_tensor (Multiply is the
                          one elementwise op neuronxcc accepts on Pool)

All membrane arithmetic is fp32 and bit-identical to the jax reference.

Per-iteration engine costs (TimelineSim model, F=2048, FD=384):
    DVE : m-stt 2253 + v-stt(384) 460 + is_le(1664) 925  = ~3640ns
    Pool: v-mult(1664) at 0.42 Q7 efficiency             = ~3400ns
    ACT : sign->u8                                       = ~1892ns
    DMA : x in 2913 + u8 spikes out 728                  = ~3641ns <- bound
All engines sit at or under the DMA floor (~233us); the previous
checkpoint (280us) was DVE-bound with both stt ops plus nothing
offloadable. The one-op u8 spike is what frees ACT to absorb nothing
and DVE to give the large v half to Pool.

Two q-tile chains are interleaved so the cross-engine hop in the Pool
half's recurrence (m -> is_le -> Pool mult -> next m) is latency-hidden:
chain-step latency ~3.2us < 2x the 3.64us iteration budget.

Sharding: x is [T=16, B=64, C=128, H=32, W=32]; the scan is elementwise
over the 8M spatial elements, so each core takes a contiguous 1/8 slice
of the flattened B*C*H*W axis viewed as [T, 128, 8192]. Spikes ship as
u8 (4x less store traffic) and widen to fp32 on the host.
"""

import numpy as np

T = 16
SPATIAL = 64 * 128 * 32 * 32  # 8388608
N_CORES = 8
NPC = SPATIAL // N_CORES      # 1048576 elements per core per timestep
P = 128                       # SBUF partitions
Q = NPC // P                  # 8192 free-dim columns per core
F = 2048                      # free-dim tile size
FD = 384                      # v columns computed on DVE; rest go to Pool
DECAY = 0.2
THRESH = 0.5

_cache = {}

# Set by test harness to request an NTFF trace / HW timing.
TRACE = False


def _build():
    from contextlib import ExitStack

    import concourse.bacc as bacc
    import concourse.tile as tile
    from concourse import mybir

    f32 = mybir.dt.float32
    u8 = mybir.dt.uint8
    Alu = mybir.AluOpType
    Act = mybir.ActivationFunctionType

    nc = bacc.Bacc("TRN2", target_bir_lowering=False, debug=False)
    x_d = nc.dram_tensor("x", [T, P, Q], f32, kind="ExternalInput").ap()
    o_d = nc.dram_tensor("spk", [T, P, Q], u8, kind="ExternalOutput").ap()

    # Register -THRESH as a const AP (like Bass.__init__ does for 0.0/1.0):
    # written once before the Tile region + barrier, so activation bias
    # reads are untracked and add no per-instruction semaphore wait (the
    # Activation ISA slot only fits one wait).
    _bias = nc.alloc_sbuf_tensor("const-f32-negthresh", [128, 1], f32)
    nc.gpsimd.memset(_bias.ap(), -THRESH)
    nc.const_aps.aps[(f32, -THRESH)] = _bias.ap()
    nc.all_engine_barrier()

    with tile.TileContext(nc) as tc, ExitStack() as ctx:
        xpool = ctx.enter_context(tc.tile_pool(name="xin", bufs=9))
        dpool = ctx.enter_context(tc.tile_pool(name="vd", bufs=5))
        ppool = ctx.enter_context(tc.tile_pool(name="vp", bufs=5))
        bpool = ctx.enter_context(tc.tile_pool(name="ble", bufs=4))
        opool = ctx.enter_context(tc.tile_pool(name="out", bufs=8))

        # Two q-tile chains interleaved per pass: while one chain's Pool
        # multiply is in flight, DVE runs the other chain.
        for pair in range(Q // (2 * F)):
            qs = [(2 * pair + i) * F for i in range(2)]
            v = {q0: (None, None) for q0 in qs}  # (v_dve, v_pool) tiles
            for t in range(T):
                for q0 in qs:
                    xt = xpool.tile([P, F], f32, name="xt")
                    nc.sync.dma_start(xt[:], x_d[t, :, q0 : q0 + F])
                    # mem update in place on the freshly-loaded x tile, one
                    # stt per carry half; at t=0, m = x[0] exactly.
                    m = xt
                    vd, vp = v[q0]
                    if vd is not None:
                        nc.vector.scalar_tensor_tensor(
                            m[:, FD:F], vp[:], DECAY, xt[:, FD:F],
                            op0=Alu.mult, op1=Alu.add,
                        )
                        nc.vector.scalar_tensor_tensor(
                            m[:, 0:FD], vd[:], DECAY, xt[:, 0:FD],
                            op0=Alu.mult, op1=Alu.add,
                        )
                    # spike in one ACT op: u8 = sign(m - 0.5), saturating
                    # write clamps -1 to 0.
                    o = opool.tile([P, F], u8, name="o")
                    nc.scalar.activation(o[:], m[:], Act.Sign, bias=-THRESH)
                    # Issued from ACT right after the sign that feeds it: the
                    # wait is pre-satisfied and SP's input prefetch stream is
                    # never head-of-line blocked by an output DMA.
                    nc.scalar.dma_start(o_d[t, :, q0 : q0 + F], o[:])
                    if t < T - 1:
                        b = bpool.tile([P, F - FD], f32, name="b")
                        nc.vector.tensor_scalar(
                            b[:], m[:, FD:F], THRESH, None, op0=Alu.is_le
                        )
                        vdn = dpool.tile([P, FD], f32, name="vdn")
                        nc.vector.scalar_tensor_tensor(
                            vdn[:], m[:, 0:FD], THRESH, m[:, 0:FD],
                            op0=Alu.is_le, op1=Alu.mult,
                        )
                        vpn = ppool.tile([P, F - FD], f32, name="vpn")
                        nc.gpsimd.tensor_tensor(
                            vpn[:], b[:], m[:, FD:F], op=Alu.mult
                        )
                        v[q0] = (vdn, vpn)
    nc.compile()
    return nc


def kernel(x: np.ndarray) -> np.ndarray:
    from concourse.bass_utils import run_bass_kernel_spmd

    if "nc" not in _cache:
        _cache["nc"] = _build()
    nc = _cache["nc"]

    x = np.ascontiguousarray(x, dtype=np.float32).reshape(T, N_CORES, NPC)
    in_maps = [
        {"x": np.ascontiguousarray(x[:, i]).reshape(T, P, Q)} for i in range(N_CORES)
    ]
    res = run_bass_kernel_spmd(
        nc, in_maps, core_ids=list(range(N_CORES)), trace=TRACE
    )
    _cache["last_results"] = res
    out = np.stack(
        [np.asarray(r["spk"]).astype(np.float32).reshape(T, NPC) for r in res.results],
        axis=1,
    )
    return out.reshape(T, 64, 128, 32, 32)


# revision 37
# speedup vs baseline: 1.2204x; 1.0089x over previous
"""LIF spike scan kernel for Trainium2 (8 NeuronCores, data-parallel).

Reference computation (per element, scanned over t):
    mem = mem * 0.2 * (1 - spk) + x[t]
    spk = (mem > 0.5)

Carry formulation (v = mem * (mem <= 0.5), the post-reset membrane):
    m   = (v * 0.2) + x[t]            -> DVE scalar_tensor_tensor (split in
                                         two column ranges, one per v half)
    spk = (m > 0.5)                   -> ONE ACT op: u8 = sign(m - 0.5);
                                         the fp32->u8 write saturates, so
                                         -1 clamps to 0 and spikes land as
                                         exact {0,1} (hardware-verified)
    v   = (m <= 0.5) * m, split:
          cols [0,FD)   : DVE scalar_tensor_tensor
          cols [FD,F)   : b = (m <= 0.5) on DVE tensor_scalar (2x mode),
                          v = b * m on Pool tensor_tensor (Multiply is the
                          one elementwise op neuronxcc accepts on Pool)

All membrane arithmetic is fp32 and bit-identical to the jax reference.

Per-iteration engine costs (TimelineSim model, F=2048, FD=384):
    DVE : m-stt 2253 + v-stt(384) 460 + is_le(1664) 925  = ~3640ns
    Pool: v-mult(1664) at 0.42 Q7 efficiency             = ~3400ns
    ACT : sign->u8                                       = ~1892ns
    DMA : x in 2913 + u8 spikes out 728                  = ~3641ns <- bound
All engines sit at or under the DMA floor (~233us); the previous
checkpoint (280us) was DVE-bound with both stt ops plus nothing
offloadable. The one-op u8 spike is what frees ACT to absorb nothing
and DVE to give the large v half to Pool.

Two q-tile chains are interleaved so the cross-engine hop in the Pool
half's recurrence (m -> is_le -> Pool mult -> next m) is latency-hidden:
chain-step latency ~3.2us < 2x the 3.64us iteration budget.

Sharding: x is [T=16, B=64, C=128, H=32, W=32]; the scan is elementwise
over the 8M spatial elements, so each core takes a contiguous 1/8 slice
of the flattened B*C*H*W axis viewed as [T, 128, 8192]. Spikes ship as
u8 (4x less store traffic) and widen to fp32 on the host.
"""

import numpy as np

T = 16
SPATIAL = 64 * 128 * 32 * 32  # 8388608
N_CORES = 8
NPC = SPATIAL // N_CORES      # 1048576 elements per core per timestep
P = 128                       # SBUF partitions
Q = NPC // P                  # 8192 free-dim columns per core
F = 2048                      # free-dim tile size
FD = 256                      # v columns computed on DVE; rest go to Pool
DECAY = 0.2
THRESH = 0.5

_cache = {}

# Set by test harness to request an NTFF trace / HW timing.
TRACE = False


def _build():
    from contextlib import ExitStack

    import concourse.bacc as bacc
    import concourse.tile as tile
    from concourse import mybir

    f32 = mybir.dt.float32
    u8 = mybir.dt.uint8
    Alu = mybir.AluOpType
    Act = mybir.ActivationFunctionType

    nc = bacc.Bacc("TRN2", target_bir_lowering=False, debug=False)
    x_d = nc.dram_tensor("x", [T, P, Q], f32, kind="ExternalInput").ap()
    o_d = nc.dram_tensor("spk", [T, P, Q], u8, kind="ExternalOutput").ap()

    # Register -THRESH as a const AP (like Bass.__init__ does for 0.0/1.0):
    # written once before the Tile region + barrier, so activation bias
    # reads are untracked and add no per-instruction semaphore wait (the
    # Activation ISA slot only fits one wait).
    _bias = nc.alloc_sbuf_tensor("const-f32-negthresh", [128, 1], f32)
    nc.gpsimd.memset(_bias.ap(), -THRESH)
    nc.const_aps.aps[(f32, -THRESH)] = _bias.ap()
    nc.all_engine_barrier()

    with tile.TileContext(nc) as tc, ExitStack() as ctx:
        xpool = ctx.enter_context(tc.tile_pool(name="xin", bufs=9))
        dpool = ctx.enter_context(tc.tile_pool(name="vd", bufs=5))
        ppool = ctx.enter_context(tc.tile_pool(name="vp", bufs=5))
        bpool = ctx.enter_context(tc.tile_pool(name="ble", bufs=4))
        opool = ctx.enter_context(tc.tile_pool(name="out", bufs=8))

        # Two q-tile chains interleaved per pass: while one chain's Pool
        # multiply is in flight, DVE runs the other chain.
        for pair in range(Q // (2 * F)):
            qs = [(2 * pair + i) * F for i in range(2)]
            v = {q0: (None, None) for q0 in qs}  # (v_dve, v_pool) tiles
            for t in range(T):
                for q0 in qs:
                    xt = xpool.tile([P, F], f32, name="xt")
                    nc.sync.dma_start(xt[:], x_d[t, :, q0 : q0 + F])
                    # mem update in place on the freshly-loaded x tile, one
                    # stt per carry half; at t=0, m = x[0] exactly.
                    m = xt
                    vd, vp = v[q0]
                    if vd is not None:
                        nc.vector.scalar_tensor_tensor(
                            m[:, FD:F], vp[:], DECAY, xt[:, FD:F],
                            op0=Alu.mult, op1=Alu.add,
                        )
                        nc.vector.scalar_tensor_tensor(
                            m[:, 0:FD], vd[:], DECAY, xt[:, 0:FD],
                            op0=Alu.mult, op1=Alu.add,
                        )
                    # spike in one ACT op: u8 = sign(m - 0.5), saturating
                    # write clamps -1 to 0.
                    o = opool.tile([P, F], u8, name="o")
                    nc.scalar.activation(o[:], m[:], Act.Sign, bias=-THRESH)
                    # Issued from ACT right after the sign that feeds it: the
                    # wait is pre-satisfied and SP's input prefetch stream is
                    # never head-of-line blocked by an output DMA.
                    nc.scalar.dma_start(o_d[t, :, q0 : q0 + F], o[:])
                    if t < T - 1:
                        b = bpool.tile([P, F - FD], f32, name="b")
                        nc.vector.tensor_scalar(
                            b[:], m[:, FD:F], THRESH, None, op0=Alu.is_le
                        )
                        vdn = dpool.tile([P, FD], f32, name="vdn")
                        nc.vector.scalar_tensor_tensor(
                            vdn[:], m[:, 0:FD], THRESH, m[:, 0:FD],
                            op0=Alu.is_le, op1=Alu.mult,
                        )
                        vpn = ppool.tile([P, F - FD], f32, name="vpn")
                        nc.gpsimd.tensor_tensor(
                            vpn[:], b[:], m[:, FD:F], op=Alu.mult
                        )
                        v[q0] = (vdn, vpn)
    nc.compile()
    return nc


def kernel(x: np.ndarray) -> np.ndarray:
    from concourse.bass_utils import run_bass_kernel_spmd

    if "nc" not in _cache:
        _cache["nc"] = _build()
    nc = _cache["nc"]

    x = np.ascontiguousarray(x, dtype=np.float32).reshape(T, N_CORES, NPC)
    in_maps = [
        {"x": np.ascontiguousarray(x[:, i]).reshape(T, P, Q)} for i in range(N_CORES)
    ]
    res = run_bass_kernel_spmd(
        nc, in_maps, core_ids=list(range(N_CORES)), trace=TRACE
    )
    _cache["last_results"] = res
    out = np.stack(
        [np.asarray(r["spk"]).astype(np.float32).reshape(T, NPC) for r in res.results],
        axis=1,
    )
    return out.reshape(T, 64, 128, 32, 32)


# revision 41
# speedup vs baseline: 1.2505x; 1.0246x over previous
"""LIF spike scan kernel for Trainium2 (8 NeuronCores, data-parallel).

Reference computation (per element, scanned over t):
    mem = mem * 0.2 * (1 - spk) + x[t]
    spk = (mem > 0.5)

Carry formulation (v = mem * (mem <= 0.5), the post-reset membrane):
    m   = (v * 0.2) + x[t]   -> DVE scalar_tensor_tensor (split per v half)
    spk = (m > 0.5)          -> ACT u8 = sign(m-0.5) (saturating write
                                clamps -1 to 0: exact {0,1}); packed tiles
                                use DVE tensor_scalar is_gt -> bf16 instead
    v, cols [0,FD)           -> DVE fused scalar_tensor_tensor
    v, cols [FD,F)           -> b = sign(0.5-m) as u8 on ACT (one op, the
                                saturating anti-mask; b=(m<0.5) exactly),
                                then v = b * m on Pool tensor_tensor (the
                                mixed u8xf32 multiply is exact on HW)

The anti-mask removes DVE's per-column comparison, dropping the
recurrence floor to ~2948ns/iter — below the packed-output DMA budget,
which re-opens output compression: q-tiles 1 and 3 pack spikes 8-per-u8
on the PE (weights W[8c+b, 128j + 32j + c] = 2^b land timestep j of a
4-step group at PSUM rows 32j..32j+15; j=0 start=True zero-resets the
region), one ACT fp32->u8 copy per group, and one full-tile [128, 2048]
u8 DMA (the only u8 DMA shape that lowers correctly). Output traffic:
2 x 4.2MB unpacked + 2 x 1.05MB packed = 10.5MB/core (vs 16.8), DMA
floor 215.6us. Tiles 0/2 stay on the single-op u8 sign spike.

Engine budget per iteration pair (one unpacked + one packed iter,
F=2048, FD=512): DVE ~3270, Pool ~3140, ACT ~3070, DMA ~3370 <- bound.

Only m=0.5-exactly deviates from the reference (b=0 there instead of 1,
measure-zero under random normal inputs; comfortably inside the 2e-2
relative tolerance). Everything else is bit-identical fp32.

Sharding: x is [T=16, B=64, C=128, H=32, W=32]; each core takes a
contiguous 1/8 of the flattened B*C*H*W axis viewed as [T, 128, 8192].
Two q-tile chains (one unpacked, one packed) interleave per pass.
"""

import numpy as np

T = 16
SPATIAL = 64 * 128 * 32 * 32  # 8388608
N_CORES = 8
NPC = SPATIAL // N_CORES      # 1048576 elements per core per timestep
P = 128                       # SBUF partitions
Q = NPC // P                  # 8192 free-dim columns per core
F = 2048                      # free-dim tile size
FD = 512                      # v columns on DVE (fused); rest via ACT+Pool
TG = 4                        # timesteps per packed PSUM group
DECAY = 0.2
THRESH = 0.5

_cache = {}

# Set by test harness to request an NTFF trace / HW timing.
TRACE = False


def _pack_weights() -> np.ndarray:
    """[128, 512] fp32, 4 blocks of [128, 128]: W[8c+b, 128j + 32j + c] =
    2^b for c in [0,16), b in [0,8); else 0. Block j lands timestep j's
    packed bytes at PSUM rows 32j..32j+15."""
    w = np.zeros((P, 4 * P), dtype=np.float32)
    for j in range(4):
        for c in range(16):
            for b in range(8):
                w[8 * c + b, P * j + 32 * j + c] = float(1 << b)
    return w


def _build():
    from contextlib import ExitStack

    import concourse.bacc as bacc
    import concourse.tile as tile
    from concourse import mybir

    f32 = mybir.dt.float32
    bf16 = mybir.dt.bfloat16
    u8 = mybir.dt.uint8
    Alu = mybir.AluOpType
    Act = mybir.ActivationFunctionType

    nc = bacc.Bacc("TRN2", target_bir_lowering=False, debug=False)
    x_d = nc.dram_tensor("x", [T, P, Q], f32, kind="ExternalInput").ap()
    w_d = nc.dram_tensor("w", [P, 4 * P], f32, kind="ExternalInput").ap()
    # Unpacked spike planes (q-tiles 0 and 2 only; packed tiles' slices
    # are never written).
    o_d = nc.dram_tensor("spk", [T, P, Q], u8, kind="ExternalOutput").ap()
    # Packed planes: [packed-tile-idx, group, 128, F]; rows 32j+c hold
    # sum_b 2^b * spk[4g+j, 8c+b, col] (rows 16..31 of each block zero).
    p_d = nc.dram_tensor("pck", [2, T // TG, P, F], u8, kind="ExternalOutput").ap()

    # Const APs for activation biases (written pre-tile-region + barrier so
    # bias reads stay untracked).
    _bn = nc.alloc_sbuf_tensor("const-f32-negthresh", [128, 1], f32)
    nc.gpsimd.memset(_bn.ap(), -THRESH)
    nc.const_aps.aps[(f32, -THRESH)] = _bn.ap()
    _bp = nc.alloc_sbuf_tensor("const-f32-posthresh", [128, 1], f32)
    nc.gpsimd.memset(_bp.ap(), THRESH)
    nc.const_aps.aps[(f32, THRESH)] = _bp.ap()
    nc.all_engine_barrier()

    with tile.TileContext(nc) as tc, ExitStack() as ctx:
        wpool = ctx.enter_context(tc.tile_pool(name="wgt", bufs=1))
        xpool = ctx.enter_context(tc.tile_pool(name="xin", bufs=8))
        dpool = ctx.enter_context(tc.tile_pool(name="vd", bufs=5))
        vpool = ctx.enter_context(tc.tile_pool(name="vp", bufs=7))
        bpool = ctx.enter_context(tc.tile_pool(name="ble", bufs=6))
        spool = ctx.enter_context(tc.tile_pool(name="spk", bufs=6))
        opool = ctx.enter_context(tc.tile_pool(name="out", bufs=6))
        ppool = ctx.enter_context(tc.tile_pool(name="acc", bufs=2, space="PSUM"))

        w_f32 = wpool.tile([P, 4 * P], f32)
        nc.sync.dma_start(w_f32[:], w_d)
        wts = []
        for j in range(TG):
            wj = wpool.tile([P, P], bf16, name=f"w{j}")
            nc.scalar.activation(wj[:], w_f32[:, P * j : P * (j + 1)], Act.Copy)
            wts.append(wj)

        pend = []  # deferred packed-group copy+DMA: (tile_idx, g, psum tile)

        def flush(final=False):
            for ti, g, pacc in pend:
                o = opool.tile([P, F], u8, name="po")
                nc.scalar.activation(o[:], pacc[:], Act.Relu)
                nc.scalar.dma_start(p_d[ti, g, :, :], o[:])
            pend.clear()

        # pair p: unpacked tile 2p (chain U) + packed tile 2p+1 (chain K)
        for pair in range(2):
            qU = (2 * pair) * F
            qK = (2 * pair + 1) * F
            v = {q0: (None, None, False) for q0 in (qU, qK)}
            acc = None
            for t in range(T):
                if t % TG == 1 and pend:
                    flush()
                if t % TG == 0:
                    acc = ppool.tile([P, F], f32, name="acc")
                for q0 in (qU, qK):
                    xt = xpool.tile([P, F], f32, name="xt")
                    nc.sync.dma_start(xt[:], x_d[t, :, q0 : q0 + F])
                    m = xt
                    vd, vp, *was_split = v[q0]
                    H = (F - FD) // 2
                    if vd is not None:
                        if was_split and was_split[0]:
                            # tail mode: consume the vp halves independently
                            # so each half of the next step launches as soon
                            # as its own Pool multiply lands.
                            nc.vector.scalar_tensor_tensor(
                                m[:, FD : FD + H], vp[:, 0:H], DECAY,
                                xt[:, FD : FD + H], op0=Alu.mult, op1=Alu.add,
                            )
                            nc.vector.scalar_tensor_tensor(
                                m[:, FD + H : F], vp[:, H:], DECAY,
                                xt[:, FD + H : F], op0=Alu.mult, op1=Alu.add,
                            )
                        else:
                            nc.vector.scalar_tensor_tensor(
                                m[:, FD:F], vp[:], DECAY, xt[:, FD:F],
                                op0=Alu.mult, op1=Alu.add,
                            )
                        nc.vector.scalar_tensor_tensor(
                            m[:, 0:FD], vd[:], DECAY, xt[:, 0:FD],
                            op0=Alu.mult, op1=Alu.add,
                        )
                    if t < T - 1:
                        # anti-mask b = (m < 0.5) in one saturating ACT op;
                        # Pool multiplies the u8 mask straight into m. In the
                        # last few timesteps (the exposed pipeline tail) the
                        # b/multiply pair is emitted per column half so the
                        # serial m->b->mult->m chain pipelines at half-width.
                        split = True
                        b = bpool.tile([P, F - FD], u8, name="b")
                        vpn = vpool.tile([P, F - FD], f32, name="vpn")
                        if split:
                            for h0, h1 in ((0, H), (H, F - FD)):
                                nc.scalar.activation(
                                    b[:, h0:h1], m[:, FD + h0 : FD + h1],
                                    Act.Sign, scale=-1.0, bias=THRESH,
                                )
                                nc.gpsimd.tensor_tensor(
                                    vpn[:, h0:h1], b[:, h0:h1],
                                    m[:, FD + h0 : FD + h1], op=Alu.mult,
                                )
                        else:
                            nc.scalar.activation(
                                b[:], m[:, FD:F], Act.Sign,
                                scale=-1.0, bias=THRESH,
                            )
                            nc.gpsimd.tensor_tensor(
                                vpn[:], b[:], m[:, FD:F], op=Alu.mult
                            )
                        vdn = dpool.tile([P, FD], f32, name="vdn")
                        nc.vector.scalar_tensor_tensor(
                            vdn[:], m[:, 0:FD], THRESH, m[:, 0:FD],
                            op0=Alu.is_le, op1=Alu.mult,
                        )
                        v[q0] = (vdn, vpn, split)
                    if q0 == qU:
                        # unpacked spike: one saturating sign -> u8
                        o = opool.tile([P, F], u8, name="o")
                        nc.scalar.activation(o[:], m[:], Act.Sign, bias=-THRESH)
                        nc.scalar.dma_start(o_d[t, :, q0 : q0 + F], o[:])
                    else:
                        # packed spike: {0,1} bf16 for the PE, then pack 128
                        # rows -> 16 u8-valued PSUM rows at block t%TG.
                        s = spool.tile([P, F], bf16, name="s")
                        nc.vector.tensor_scalar(
                            s[:], m[:], THRESH, None, op0=Alu.is_gt
                        )
                        j = t % TG
                        for c in range(F // 512):
                            ch = slice(512 * c, 512 * (c + 1))
                            nc.tensor.matmul(
                                acc[:, ch], wts[j][:], s[:, ch],
                                start=(j == 0), stop=(j == TG - 1),
                                skip_group_check=True,
                            )
                        if j == TG - 1:
                            pend.append((pair, t // TG, acc))
            flush(final=True)
    nc.compile()
    return nc


def kernel(x: np.ndarray) -> np.ndarray:
    from concourse.bass_utils import run_bass_kernel_spmd

    if "nc" not in _cache:
        _cache["nc"] = _build()
    nc = _cache["nc"]

    x = np.ascontiguousarray(x, dtype=np.float32).reshape(T, N_CORES, NPC)
    w = _pack_weights()
    in_maps = [
        {"x": np.ascontiguousarray(x[:, i]).reshape(T, P, Q), "w": w}
        for i in range(N_CORES)
    ]
    res = run_bass_kernel_spmd(
        nc, in_maps, core_ids=list(range(N_CORES)), trace=TRACE
    )
    _cache["last_results"] = res
    outs = []
    for r in res.results:
        spk = np.asarray(r["spk"]).reshape(T, P, Q).copy()
        pck = np.asarray(r["pck"]).reshape(2, T // TG, P, F)
        for pi in range(2):
            tile0 = (2 * pi + 1) * F
            blk = pck[pi].reshape(T // TG, TG, 32, F)[:, :, 0:16, :]
            bits = np.unpackbits(blk[:, :, :, None, :], axis=3, bitorder="little")
            spk[:, :, tile0 : tile0 + F] = bits.reshape(T, P, F)
        outs.append(spk)
    out = np.stack(outs, axis=1).astype(np.float32).reshape(T, NPC * N_CORES)
    return out.reshape(T, 64, 128, 32, 32)
